# revision 1
# baseline (speedup 1.0000x reference)
"""Trainium2 Bass kernel for 2-layer GCN + 2-step propagation + log_softmax.

Strategy (8 NeuronCores, SPMD):
  - Nodes row-sharded: core c owns srcs [12500c, 12500(c+1)).
  - MLP (x @ W1 -> relu -> @ W2) on the tensor engine, producing the
    class-transposed logits shard LT [16, 12544] per core (DRAM).
  - AllGather LT shards -> full transposed logits table [128, 12544] where
    partition 16j+c holds class c of core j's node chunk.  This is exactly
    the per-GPSIMD-core chunked table layout ap_gather needs.
  - SpMM (agg[s] = sum_{(s,d) in E} logits[d]) per iteration:
      * shard edges bucketed by dst chunk j (the gpsimd core group that holds
        that chunk), sorted by src within the bucket;
      * ap_gather fetches logitsT[dst] for every edge (classes land on the
        16 partitions of group j);
      * tensor_tensor_scan computes a running cumsum along the edge stream;
      * a second (small) ap_gather extracts the cumsum at host-precomputed
        segment boundaries; adjacent differences give per-(group, src) sums;
      * a PE matmul with a stacked identity folds the 8 groups' partial sums
        into agg^T [16, n_srcs];
      * logits1T = scale * aggT + alpha * LT  (DVE), streamed to DRAM and
        AllGathered again for the second iteration.
  - log_softmax via PE transpose back to node-major tiles + ACT exp/ln.
"""

import sys

sys.path.insert(0, "/opt/trn_rl_repo")

import numpy as np

_COMPILED_CACHE = {}

N_NODES = 100000
N_FEAT = 512
HIDDEN = 128
N_CLASS = 16
N_EDGES = 3200000
ALPHA = 0.25
N_ITERS = 2

NCORES = 8
SHARD = N_NODES // NCORES          # 12500
SHARD_PAD = 12544                  # multiple of 128
N_TILES = 8                        # src tiles per shard per iteration
TILE_SRCS = SHARD_PAD // N_TILES   # 1568 srcs per tile
P = 128


def _host_prep(edge_src, edge_dst):
    """Per-core gather index arrays + boundary arrays (same for both iters)."""
    es = np.asarray(edge_src).astype(np.int64)
    ed = np.asarray(edge_dst).astype(np.int64)

    deg_full = np.bincount(es, minlength=N_NODES).astype(np.float32)

    per_core = []
    core_of = es // SHARD
    grp = ed // SHARD               # dst chunk = gpsimd group
    dst_local = ed - grp * SHARD    # 0..12499, < 12544 table elems

    for c in range(NCORES):
        m = core_of == c
        s_loc = es[m] - c * SHARD
        g = grp[m]
        dl = dst_local[m]

        tile_of = s_loc // TILE_SRCS
        order = np.lexsort((s_loc, g, tile_of))
        s_loc, g, dl, tile_of = s_loc[order], g[order], dl[order], tile_of[order]

        idx_tiles, bnd_tiles, nidx_list, nb_list = [], [], [], []
        for t in range(N_TILES):
            tm = tile_of == t
            gt, st, dt = g[tm], s_loc[tm], dl[tm]
            s_base = t * TILE_SRCS
            group_lists, group_bounds = [], []
            maxn = 0
            for j in range(NCORES):
                jm = gt == j
                dj = dt[jm]
                cnt = np.bincount(st[jm] - s_base, minlength=TILE_SRCS)
                bounds = np.concatenate([[0], np.cumsum(cnt)])
                group_lists.append(dj)
                group_bounds.append(bounds)
                maxn = max(maxn, len(dj) + 1)
            nidx = -(-max(maxn, 128) // 128) * 128
            nb = -(-(TILE_SRCS + 1) // 256) * 256
            idx_arr = np.full((P, nidx // 16), SHARD, dtype=np.int16)
            bnd_arr = np.zeros((P, nb // 16), dtype=np.int16)
            for j in range(NCORES):
                dj = group_lists[j]
                # slot 0 is a dummy zero-gather so the in-place inclusive
                # cumsum C'[k] equals the exclusive cumsum of the real edges;
                # boundary positions then index C' directly (C'[0] = 0).
                lst = np.full(nidx, SHARD, dtype=np.int64)
                lst[1: len(dj) + 1] = dj
                idx_arr[16 * j:16 * j + 16, :] = (
                    lst.reshape(nidx // 16, 16).T.astype(np.int16))
                bl = np.zeros(nb, dtype=np.int64)
                bl[: TILE_SRCS + 1] = group_bounds[j]
                bnd_arr[16 * j:16 * j + 16, :] = (
                    bl.reshape(nb // 16, 16).T.astype(np.int16))
            idx_tiles.append(idx_arr)
            bnd_tiles.append(bnd_arr)
            nidx_list.append(nidx)
            nb_list.append(nb)

        per_core.append(dict(idx_tiles=idx_tiles, bnd_tiles=bnd_tiles,
                             nidx_list=nidx_list, nb_list=nb_list))

    return deg_full, per_core


def _chunks_of(total, size):
    out = []
    q = 0
    while q < total:
        out.append((q, min(size, total - q)))
        q += size
    return out


def _build_program(nidx_list, nb_list):
    import concourse.bass as bass
    import concourse.tile as tile
    import concourse.mybir as mybir
    from concourse import bacc

    f32 = mybir.dt.float32
    i16 = mybir.dt.int16

    nc = bacc.Bacc("TRN2", target_bir_lowering=False, debug=False,
                   num_devices=NCORES)

    # ---- I/O ----
    x_in = nc.dram_tensor("x_shard", [SHARD_PAD, N_FEAT], f32,
                          kind="ExternalInput").ap()
    w1_in = nc.dram_tensor("w1", [N_FEAT, HIDDEN], f32,
                           kind="ExternalInput").ap()
    w2_in = nc.dram_tensor("w2", [HIDDEN, N_CLASS], f32,
                           kind="ExternalInput").ap()
    deg_in = nc.dram_tensor("deg16", [16, SHARD_PAD], f32,
                            kind="ExternalInput").ap()
    e16_in = nc.dram_tensor("e16", [P, 16], f32, kind="ExternalInput").ap()
    ident_in = nc.dram_tensor("ident", [P, P], f32, kind="ExternalInput").ap()
    sum_nidx = sum(nidx_list)
    sum_nb = sum(nb_list)
    idx_in = nc.dram_tensor("gidx", [P, sum_nidx // 16], i16,
                            kind="ExternalInput").ap()
    bnd_in = nc.dram_tensor("gbnd", [P, sum_nb // 16], i16,
                            kind="ExternalInput").ap()
    out_ext = nc.dram_tensor("out", [SHARD_PAD, N_CLASS], f32,
                             kind="ExternalOutput").ap()

    # ---- internal DRAM ----
    lt_dram = nc.dram_tensor("lt_shard", [16, SHARD_PAD], f32)
    new_dram = nc.dram_tensor("newlog", [16, SHARD_PAD], f32)
    log2_dram = nc.dram_tensor("log2", [16, SHARD_PAD], f32)
    tbl_dram = [
        nc.dram_tensor(f"tbl{i}", [P * SHARD_PAD], f32, addr_space="Shared")
        for i in range(2)
    ]

    NT128 = SHARD_PAD // P  # 98 node tiles

    with tile.TileContext(nc) as tc:
        with (
            tc.tile_pool(name="persist", bufs=1) as pp,
            tc.tile_pool(name="mlp", bufs=3) as mp,
            tc.tile_pool(name="mlp_ps", bufs=2, space="PSUM") as mps,
            tc.tile_pool(name="sp1", bufs=1) as sp1,
            tc.tile_pool(name="sp2", bufs=2) as sp2,
            tc.tile_pool(name="chk", bufs=2) as ck,
            tc.tile_pool(name="sp_ps", bufs=2, space="PSUM") as sps,
        ):
            # ---------- constants ----------
            w1_sb = pp.tile([P, 4 * HIDDEN], f32)
            for k in range(4):
                nc.sync.dma_start(w1_sb[:, k * HIDDEN:(k + 1) * HIDDEN],
                                  w1_in[k * P:(k + 1) * P, :])
            w2_sb = pp.tile([P, N_CLASS], f32)
            nc.sync.dma_start(w2_sb[:, :], w2_in)
            e16_sb = pp.tile([P, 16], f32)
            nc.sync.dma_start(e16_sb[:, :], e16_in)
            ident_sb = pp.tile([P, P], f32)
            nc.sync.dma_start(ident_sb[:, :], ident_in)
            gidx_sb = pp.tile([P, sum_nidx // 16], i16)
            nc.sync.dma_start(gidx_sb[:, :], idx_in)
            gbnd_sb = pp.tile([P, sum_nb // 16], i16)
            nc.sync.dma_start(gbnd_sb[:, :], bnd_in)

            # ---------- MLP ----------
            for t in range(NT128):
                xt = mp.tile([P, N_FEAT], f32, tag="xt")
                nc.sync.dma_start(xt[:, :], x_in[t * P:(t + 1) * P, :])
                hps = mps.tile([P, P], f32, tag="hps")
                for k in range(4):
                    xps = mps.tile([P, P], f32, tag="t1")
                    nc.tensor.transpose(out=xps[:, :],
                                        in_=xt[:, k * P:(k + 1) * P],
                                        identity=ident_sb[:, :])
                    xTs = mp.tile([P, P], f32, tag="xTs")
                    nc.vector.tensor_copy(xTs[:, :], xps[:, :])
                    nc.tensor.matmul(out=hps[:, :],
                                     lhsT=w1_sb[:, k * HIDDEN:(k + 1) * HIDDEN],
                                     rhs=xTs[:, :],
                                     start=(k == 0), stop=(k == 3))
                hT = mp.tile([P, P], f32, tag="hT")
                nc.scalar.activation(hT[:, :], hps[:, :],
                                     mybir.ActivationFunctionType.Relu)
                lps = mps.tile([16, P], f32, tag="lps")
                nc.tensor.matmul(out=lps[:, :], lhsT=w2_sb[:, :], rhs=hT[:, :],
                                 start=True, stop=True)
                ltc = mp.tile([16, P], f32, tag="ltc")
                nc.vector.tensor_copy(ltc[:, :], lps[:, :])
                nc.sync.dma_start(lt_dram[:, t * P:(t + 1) * P], ltc[:, :])

            nc.gpsimd.collective_compute(
                "AllGather", mybir.AluOpType.bypass,
                replica_groups=[list(range(NCORES))],
                ins=[lt_dram.ap()], outs=[tbl_dram[0].ap()],
            )

            # ---------- propagation iterations ----------
            for it in range(N_ITERS):
                tbl_sb = sp1.tile([P, SHARD_PAD], f32, tag="tbl",
                                  name=f"tbl_{it}")
                nc.sync.dma_start(
                    tbl_sb[:, :],
                    tbl_dram[it].ap().rearrange("(p n) -> p n", p=P))

                dst_dram = log2_dram if it == N_ITERS - 1 else new_dram

                off_i = 0
                off_b = 0
                for t in range(N_TILES):
                    nidx = nidx_list[t]
                    nb = nb_list[t]
                    gbuf = sp2.tile([P, nidx], f32, tag="gbuf",
                                    name=f"gbuf_{it}_{t}")
                    nc.gpsimd.ap_gather(
                        out_ap=gbuf[:, :].rearrange("p (n d) -> p n d", d=1),
                        in_ap=tbl_sb[:, :].rearrange("p (n d) -> p n d", d=1),
                        idxs_ap=gidx_sb[:, off_i:off_i + nidx // 16],
                        channels=P, num_elems=SHARD_PAD, d=1, num_idxs=nidx,
                    )
                    nc.vector.tensor_tensor_scan(
                        out=gbuf[:, :],
                        data0=gbuf[:, :], data1=gbuf[:, :],
                        initial=0.0,
                        op0=mybir.AluOpType.add,
                        op1=mybir.AluOpType.bypass,
                    )
                    ext = sp2.tile([P, nb], f32, tag="ext",
                                   name=f"ext_{it}_{t}")
                    nc.gpsimd.ap_gather(
                        out_ap=ext[:, :].rearrange("p (n d) -> p n d", d=1),
                        in_ap=gbuf[:, :].rearrange("p (n d) -> p n d", d=1),
                        idxs_ap=gbnd_sb[:, off_b:off_b + nb // 16],
                        channels=P, num_elems=nidx, d=1, num_idxs=nb,
                    )
                    part = sp2.tile([P, TILE_SRCS], f32, tag="part",
                                    name=f"part_{it}_{t}")
                    nc.vector.tensor_tensor(
                        out=part[:, :],
                        in0=ext[:, 1:TILE_SRCS + 1],
                        in1=ext[:, 0:TILE_SRCS],
                        op=mybir.AluOpType.subtract,
                    )
                    for (q0, cw) in _chunks_of(TILE_SRCS, 512):
                        aps = sps.tile([16, 512], f32, tag="aps")
                        nc.tensor.matmul(out=aps[:, :cw], lhsT=e16_sb[:, :],
                                         rhs=part[:, q0:q0 + cw],
                                         start=True, stop=True)
                        col = t * TILE_SRCS + q0
                        sl = slice(col, col + cw)
                        degc = ck.tile([16, 512], f32, tag="degc")
                        nc.sync.dma_start(degc[:, :cw], deg_in[:, sl])
                        scc = ck.tile([16, 512], f32, tag="scc")
                        nc.vector.tensor_scalar_max(scc[:, :cw], degc[:, :cw],
                                                    1e-12)
                        nc.vector.reciprocal(scc[:, :cw], scc[:, :cw])
                        nc.vector.tensor_scalar_mul(scc[:, :cw], scc[:, :cw],
                                                    1.0 - ALPHA)
                        ltk = ck.tile([16, 512], f32, tag="ltk")
                        nc.sync.dma_start(ltk[:, :cw], lt_dram[:, sl])
                        tmp = ck.tile([16, 512], f32, tag="tmp")
                        nc.vector.tensor_mul(tmp[:, :cw], aps[:, :cw],
                                             scc[:, :cw])
                        outc = ck.tile([16, 512], f32, tag="outc")
                        nc.vector.scalar_tensor_tensor(
                            out=outc[:, :cw],
                            in0=ltk[:, :cw],
                            scalar=ALPHA,
                            in1=tmp[:, :cw],
                            op0=mybir.AluOpType.mult,
                            op1=mybir.AluOpType.add,
                        )
                        nc.sync.dma_start(dst_dram[:, sl], outc[:, :cw])
                    off_i += nidx // 16
                    off_b += nb // 16

                if it < N_ITERS - 1:
                    nc.gpsimd.collective_compute(
                        "AllGather", mybir.AluOpType.bypass,
                        replica_groups=[list(range(NCORES))],
                        ins=[new_dram.ap()], outs=[tbl_dram[1].ap()],
                    )

            # ---------- log_softmax + output ----------
            for t in range(NT128):
                l2c = mp.tile([16, P], f32, tag="l2c")
                nc.sync.dma_start(l2c[:, :], log2_dram[:, t * P:(t + 1) * P])
                tps = mps.tile([P, 16], f32, tag="t1")
                nc.tensor.transpose(out=tps[:, :], in_=l2c[:, :],
                                    identity=ident_sb[:16, :16])
                row = mp.tile([P, 16], f32, tag="row")
                nc.vector.tensor_copy(row[:, :], tps[:, :])
                mx = mp.tile([P, 1], f32, tag="mx")
                nc.vector.tensor_reduce(out=mx[:, :], in_=row[:, :],
                                        axis=mybir.AxisListType.X,
                                        op=mybir.AluOpType.max)
                shifted = mp.tile([P, 16], f32, tag="shifted")
                nc.vector.tensor_tensor(out=shifted[:, :], in0=row[:, :],
                                        in1=mx[:, :].to_broadcast([P, 16]),
                                        op=mybir.AluOpType.subtract)
                ex = mp.tile([P, 16], f32, tag="ex")
                nc.scalar.activation(ex[:, :], shifted[:, :],
                                     mybir.ActivationFunctionType.Exp)
                sm = mp.tile([P, 1], f32, tag="mx")
                nc.vector.tensor_reduce(out=sm[:, :], in_=ex[:, :],
                                        axis=mybir.AxisListType.X,
                                        op=mybir.AluOpType.add)
                lg = mp.tile([P, 1], f32, tag="mx")
                nc.scalar.activation(lg[:, :], sm[:, :],
                                     mybir.ActivationFunctionType.Ln)
                res = mp.tile([P, 16], f32, tag="res")
                nc.vector.tensor_tensor(out=res[:, :], in0=shifted[:, :],
                                        in1=lg[:, :].to_broadcast([P, 16]),
                                        op=mybir.AluOpType.subtract)
                nc.sync.dma_start(out_ext[t * P:(t + 1) * P, :], res[:, :])

    nc.compile()
    return nc


def kernel(x, W1, W2, edge_src, edge_dst):
    from concourse.bass_utils import run_bass_kernel_spmd

    x = np.asarray(x, dtype=np.float32)
    W1 = np.asarray(W1, dtype=np.float32)
    W2 = np.asarray(W2, dtype=np.float32)

    deg_full, per_core = _host_prep(edge_src, edge_dst)

    nidx_list = [max(pc["nidx_list"][t] for pc in per_core)
                 for t in range(N_TILES)]
    nb_list = [max(pc["nb_list"][t] for pc in per_core) for t in range(N_TILES)]

    key = (tuple(nidx_list), tuple(nb_list))
    if key not in _COMPILED_CACHE:
        _COMPILED_CACHE[key] = _build_program(nidx_list, nb_list)
    nc = _COMPILED_CACHE[key]

    e16 = np.tile(np.eye(16, dtype=np.float32), (8, 1))
    ident = np.eye(P, dtype=np.float32)

    in_maps = []
    for c in range(NCORES):
        pc = per_core[c]
        x_shard = np.zeros((SHARD_PAD, N_FEAT), dtype=np.float32)
        x_shard[:SHARD] = x[c * SHARD:(c + 1) * SHARD]
        deg16 = np.tile(
            np.pad(deg_full[c * SHARD:(c + 1) * SHARD],
                   (0, SHARD_PAD - SHARD), constant_values=1.0)[None, :],
            (16, 1)).astype(np.float32)
        idx_cat = np.concatenate(
            [np.pad(pc["idx_tiles"][t],
                    ((0, 0), (0, (nidx_list[t] - pc["nidx_list"][t]) // 16)),
                    constant_values=SHARD)
             for t in range(N_TILES)], axis=1)
        bnd_cat = np.concatenate(
            [np.pad(pc["bnd_tiles"][t],
                    ((0, 0), (0, (nb_list[t] - pc["nb_list"][t]) // 16)))
             for t in range(N_TILES)], axis=1)
        in_maps.append({
            "x_shard": x_shard,
            "w1": W1,
            "w2": W2,
            "deg16": deg16,
            "e16": e16,
            "ident": ident,
            "gidx": idx_cat,
            "gbnd": bnd_cat,
        })

    res = run_bass_kernel_spmd(nc, in_maps, list(range(NCORES)))

    out = np.empty((N_NODES, N_CLASS), dtype=np.float32)
    for c in range(NCORES):
        out[c * SHARD:(c + 1) * SHARD] = res.results[c]["out"][:SHARD]
    return out



# revision 2
# speedup vs baseline: 2.8315x; 2.8315x over previous
"""Trainium2 Bass kernel for 2-layer GCN + 2-step propagation + log_softmax.

Strategy (8 NeuronCores, SPMD):
  - Nodes row-sharded: core c owns srcs [12500c, 12500(c+1)).
  - MLP on the tensor engine from a host-pretransposed x shard (no PE
    transposes): hT[128,nodes] = W1.T @ xT accumulated over 4 feature
    blocks, relu, then logitsT [16, nodes] = W2.T @ hT.
  - AllGather logitsT shards -> full transposed table [128, 12544] where
    partition 16j+c holds class c of core j's node chunk (the layout
    ap_gather needs).
  - SpMM per iteration: edges bucketed by dst chunk, sorted by src;
    ap_gather fetches logitsT[dst]; tensor_tensor_scan cumsums the edge
    stream; a small ap_gather extracts segment boundaries; adjacent
    diffs give per-(group, src) sums; a PE matmul with stacked identity
    folds the 8 groups into aggT; update = scale*aggT + alpha*LT with a
    host-precomputed scale table kept in SBUF.
  - log_softmax: PE-transpose tiles to node-major, batched exp/ln (two
    activation table loads total instead of two per tile).
"""

import sys

sys.path.insert(0, "/opt/trn_rl_repo")

import numpy as np

_COMPILED_CACHE = {}

N_NODES = 100000
N_FEAT = 512
HIDDEN = 128
N_CLASS = 16
N_EDGES = 3200000
ALPHA = 0.25
N_ITERS = 2

NCORES = 8
SHARD = N_NODES // NCORES          # 12500
SHARD_PAD = 12544                  # multiple of 128
N_TILES = 8                        # src tiles per shard per iteration
TILE_SRCS = SHARD_PAD // N_TILES   # 1568 srcs per tile
P = 128
MLP_CHUNK = 512                    # nodes per MLP chunk (PSUM bank = 512 f32)


def _host_prep(edge_src, edge_dst):
    """Per-core gather index arrays + boundary arrays (same for both iters)."""
    es = np.asarray(edge_src).astype(np.int64)
    ed = np.asarray(edge_dst).astype(np.int64)

    deg_full = np.bincount(es, minlength=N_NODES).astype(np.float32)

    per_core = []
    core_of = es // SHARD
    grp = ed // SHARD               # dst chunk = gpsimd group
    dst_local = ed - grp * SHARD    # 0..12499, < 12544 table elems

    for c in range(NCORES):
        m = core_of == c
        s_loc = es[m] - c * SHARD
        g = grp[m]
        dl = dst_local[m]

        tile_of = s_loc // TILE_SRCS
        order = np.lexsort((s_loc, g, tile_of))
        s_loc, g, dl, tile_of = s_loc[order], g[order], dl[order], tile_of[order]

        idx_tiles, bnd_tiles, nidx_list, nb_list = [], [], [], []
        for t in range(N_TILES):
            tm = tile_of == t
            gt, st, dt = g[tm], s_loc[tm], dl[tm]
            s_base = t * TILE_SRCS
            group_lists, group_bounds = [], []
            maxn = 0
            for j in range(NCORES):
                jm = gt == j
                dj = dt[jm]
                cnt = np.bincount(st[jm] - s_base, minlength=TILE_SRCS)
                bounds = np.concatenate([[0], np.cumsum(cnt)])
                group_lists.append(dj)
                group_bounds.append(bounds)
                maxn = max(maxn, len(dj) + 1)
            nidx = -(-max(maxn, 128) // 128) * 128
            nb = -(-(TILE_SRCS + 1) // 256) * 256
            idx_arr = np.full((P, nidx // 16), SHARD, dtype=np.int16)
            bnd_arr = np.zeros((P, nb // 16), dtype=np.int16)
            for j in range(NCORES):
                dj = group_lists[j]
                # slot 0 is a dummy zero-gather so the in-place inclusive
                # cumsum C'[k] equals the exclusive cumsum of the real edges;
                # boundary positions then index C' directly (C'[0] = 0).
                lst = np.full(nidx, SHARD, dtype=np.int64)
                lst[1: len(dj) + 1] = dj
                idx_arr[16 * j:16 * j + 16, :] = (
                    lst.reshape(nidx // 16, 16).T.astype(np.int16))
                bl = np.zeros(nb, dtype=np.int64)
                bl[: TILE_SRCS + 1] = group_bounds[j]
                bnd_arr[16 * j:16 * j + 16, :] = (
                    bl.reshape(nb // 16, 16).T.astype(np.int16))
            idx_tiles.append(idx_arr)
            bnd_tiles.append(bnd_arr)
            nidx_list.append(nidx)
            nb_list.append(nb)

        per_core.append(dict(idx_tiles=idx_tiles, bnd_tiles=bnd_tiles,
                             nidx_list=nidx_list, nb_list=nb_list))

    return deg_full, per_core


def _chunks_of(total, size):
    out = []
    q = 0
    while q < total:
        out.append((q, min(size, total - q)))
        q += size
    return out


def _build_program(nidx_list, nb_list):
    import concourse.bass as bass
    import concourse.tile as tile
    import concourse.mybir as mybir
    from concourse import bacc

    f32 = mybir.dt.float32
    i16 = mybir.dt.int16

    nc = bacc.Bacc("TRN2", target_bir_lowering=False, debug=False,
                   num_devices=NCORES)

    # ---- I/O ----
    xt_in = nc.dram_tensor("xt_shard", [N_FEAT, SHARD_PAD], f32,
                           kind="ExternalInput").ap()
    w1_in = nc.dram_tensor("w1", [N_FEAT, HIDDEN], f32,
                           kind="ExternalInput").ap()
    w2_in = nc.dram_tensor("w2", [HIDDEN, N_CLASS], f32,
                           kind="ExternalInput").ap()
    scale_in = nc.dram_tensor("scale16", [16, SHARD_PAD], f32,
                              kind="ExternalInput").ap()
    e16_in = nc.dram_tensor("e16", [P, 16], f32, kind="ExternalInput").ap()
    ident_in = nc.dram_tensor("ident", [16, 16], f32, kind="ExternalInput").ap()
    sum_nidx = sum(nidx_list)
    sum_nb = sum(nb_list)
    idx_in = nc.dram_tensor("gidx", [P, sum_nidx // 16], i16,
                            kind="ExternalInput").ap()
    bnd_in = nc.dram_tensor("gbnd", [P, sum_nb // 16], i16,
                            kind="ExternalInput").ap()
    out_ext = nc.dram_tensor("out", [SHARD_PAD, N_CLASS], f32,
                             kind="ExternalOutput").ap()

    # ---- internal DRAM ----
    lt_dram = nc.dram_tensor("lt_shard", [16, SHARD_PAD], f32)
    new_dram = nc.dram_tensor("newlog", [16, SHARD_PAD], f32)
    tbl_dram = [
        nc.dram_tensor(f"tbl{i}", [P * SHARD_PAD], f32, addr_space="Shared")
        for i in range(2)
    ]

    NT128 = SHARD_PAD // P  # 98 node tiles

    with tile.TileContext(nc) as tc:
        with (
            tc.tile_pool(name="persist", bufs=1) as pp,
            tc.tile_pool(name="mlp", bufs=2) as mp,
            tc.tile_pool(name="mlp_ps", bufs=2, space="PSUM") as mps,
            tc.tile_pool(name="sp1", bufs=1) as sp1,
            tc.tile_pool(name="sp2", bufs=2) as sp2,
            tc.tile_pool(name="chk", bufs=2) as ck,
            tc.tile_pool(name="sp_ps", bufs=2, space="PSUM") as sps,
            tc.tile_pool(name="sm", bufs=2) as smp,
            tc.tile_pool(name="sm_ps", bufs=2, space="PSUM") as smps,
        ):
            # ---------- constants ----------
            w1_sb = pp.tile([P, 4 * HIDDEN], f32)
            for k in range(4):
                nc.sync.dma_start(w1_sb[:, k * HIDDEN:(k + 1) * HIDDEN],
                                  w1_in[k * P:(k + 1) * P, :])
            w2_sb = pp.tile([P, N_CLASS], f32)
            nc.sync.dma_start(w2_sb[:, :], w2_in)
            e16_sb = pp.tile([P, 16], f32)
            nc.sync.dma_start(e16_sb[:, :], e16_in)
            ident_sb = pp.tile([16, 16], f32)
            nc.sync.dma_start(ident_sb[:, :], ident_in)
            scale_sb = pp.tile([16, SHARD_PAD], f32)
            nc.sync.dma_start(scale_sb[:, :], scale_in)
            gidx_sb = pp.tile([P, sum_nidx // 16], i16)
            nc.sync.dma_start(gidx_sb[:, :], idx_in)
            gbnd_sb = pp.tile([P, sum_nb // 16], i16)
            nc.sync.dma_start(gbnd_sb[:, :], bnd_in)
            lt_sb = pp.tile([16, SHARD_PAD], f32)      # local logits, persistent
            log2_sb = pp.tile([16, SHARD_PAD], f32)    # final logits, persistent

            # ---------- MLP ----------
            for (q0, cw) in _chunks_of(SHARD_PAD, MLP_CHUNK):
                hps = mps.tile([P, MLP_CHUNK], f32, tag="hps")
                for k in range(4):
                    xc = mp.tile([P, MLP_CHUNK], f32, tag=f"xc{k}")
                    nc.sync.dma_start(xc[:, :cw],
                                      xt_in[k * P:(k + 1) * P, q0:q0 + cw])
                    nc.tensor.matmul(out=hps[:, :cw],
                                     lhsT=w1_sb[:, k * HIDDEN:(k + 1) * HIDDEN],
                                     rhs=xc[:, :cw],
                                     start=(k == 0), stop=(k == 3))
                hT = mp.tile([P, MLP_CHUNK], f32, tag="hT")
                nc.scalar.activation(hT[:, :cw], hps[:, :cw],
                                     mybir.ActivationFunctionType.Relu)
                lps = mps.tile([16, MLP_CHUNK], f32, tag="lps")
                nc.tensor.matmul(out=lps[:, :cw], lhsT=w2_sb[:, :],
                                 rhs=hT[:, :cw], start=True, stop=True)
                nc.vector.tensor_copy(lt_sb[:, q0:q0 + cw], lps[:, :cw])
                nc.sync.dma_start(lt_dram[:, q0:q0 + cw], lt_sb[:, q0:q0 + cw])

            nc.gpsimd.collective_compute(
                "AllGather", mybir.AluOpType.bypass,
                replica_groups=[list(range(NCORES))],
                ins=[lt_dram.ap()], outs=[tbl_dram[0].ap()],
            )

            # ---------- propagation iterations ----------
            for it in range(N_ITERS):
                tbl_sb = sp1.tile([P, SHARD_PAD], f32, tag="tbl",
                                  name=f"tbl_{it}")
                nc.sync.dma_start(
                    tbl_sb[:, :],
                    tbl_dram[it].ap().rearrange("(p n) -> p n", p=P))

                off_i = 0
                off_b = 0
                for t in range(N_TILES):
                    nidx = nidx_list[t]
                    nb = nb_list[t]
                    gbuf = sp2.tile([P, nidx], f32, tag="gbuf",
                                    name=f"gbuf_{it}_{t}")
                    nc.gpsimd.ap_gather(
                        out_ap=gbuf[:, :].rearrange("p (n d) -> p n d", d=1),
                        in_ap=tbl_sb[:, :].rearrange("p (n d) -> p n d", d=1),
                        idxs_ap=gidx_sb[:, off_i:off_i + nidx // 16],
                        channels=P, num_elems=SHARD_PAD, d=1, num_idxs=nidx,
                    )
                    nc.vector.tensor_tensor_scan(
                        out=gbuf[:, :],
                        data0=gbuf[:, :], data1=gbuf[:, :],
                        initial=0.0,
                        op0=mybir.AluOpType.add,
                        op1=mybir.AluOpType.bypass,
                    )
                    ext = sp2.tile([P, nb], f32, tag="ext",
                                   name=f"ext_{it}_{t}")
                    nc.gpsimd.ap_gather(
                        out_ap=ext[:, :].rearrange("p (n d) -> p n d", d=1),
                        in_ap=gbuf[:, :].rearrange("p (n d) -> p n d", d=1),
                        idxs_ap=gbnd_sb[:, off_b:off_b + nb // 16],
                        channels=P, num_elems=nidx, d=1, num_idxs=nb,
                    )
                    part = sp2.tile([P, TILE_SRCS], f32, tag="part",
                                    name=f"part_{it}_{t}")
                    nc.vector.tensor_tensor(
                        out=part[:, :],
                        in0=ext[:, 1:TILE_SRCS + 1],
                        in1=ext[:, 0:TILE_SRCS],
                        op=mybir.AluOpType.subtract,
                    )
                    for (q0, cw) in _chunks_of(TILE_SRCS, 512):
                        aps = sps.tile([16, 512], f32, tag="aps")
                        nc.tensor.matmul(out=aps[:, :cw], lhsT=e16_sb[:, :],
                                         rhs=part[:, q0:q0 + cw],
                                         start=True, stop=True)
                        col = t * TILE_SRCS + q0
                        sl = slice(col, col + cw)
                        tmp = ck.tile([16, 512], f32, tag="tmp")
                        nc.vector.tensor_mul(tmp[:, :cw], aps[:, :cw],
                                             scale_sb[:, sl])
                        if it < N_ITERS - 1:
                            outc = ck.tile([16, 512], f32, tag="outc")
                            nc.vector.scalar_tensor_tensor(
                                out=outc[:, :cw],
                                in0=lt_sb[:, sl],
                                scalar=ALPHA,
                                in1=tmp[:, :cw],
                                op0=mybir.AluOpType.mult,
                                op1=mybir.AluOpType.add,
                            )
                            nc.sync.dma_start(new_dram[:, sl], outc[:, :cw])
                        else:
                            nc.vector.scalar_tensor_tensor(
                                out=log2_sb[:, sl],
                                in0=lt_sb[:, sl],
                                scalar=ALPHA,
                                in1=tmp[:, :cw],
                                op0=mybir.AluOpType.mult,
                                op1=mybir.AluOpType.add,
                            )
                    off_i += nidx // 16
                    off_b += nb // 16

                if it < N_ITERS - 1:
                    nc.gpsimd.collective_compute(
                        "AllGather", mybir.AluOpType.bypass,
                        replica_groups=[list(range(NCORES))],
                        ins=[new_dram.ap()], outs=[tbl_dram[1].ap()],
                    )

            # ---------- log_softmax (batched exp/ln) ----------
            row_sb = pp.tile([P, NT128 * 16], f32)
            mx_sb = pp.tile([P, NT128], f32)
            sh_sb = pp.tile([P, NT128 * 16], f32)
            sm_sb = pp.tile([P, NT128], f32)
            lg_sb = pp.tile([P, NT128], f32)
            for t in range(NT128):
                tps = smps.tile([P, 16], f32, tag="tps")
                nc.tensor.transpose(out=tps[:, :],
                                    in_=log2_sb[:, t * P:(t + 1) * P],
                                    identity=ident_sb[:, :])
                nc.scalar.activation(row_sb[:, t * 16:(t + 1) * 16], tps[:, :],
                                     mybir.ActivationFunctionType.Copy)
            for t in range(NT128):
                nc.vector.tensor_reduce(out=mx_sb[:, t:t + 1],
                                        in_=row_sb[:, t * 16:(t + 1) * 16],
                                        axis=mybir.AxisListType.X,
                                        op=mybir.AluOpType.max)
                nc.vector.tensor_tensor(
                    out=sh_sb[:, t * 16:(t + 1) * 16],
                    in0=row_sb[:, t * 16:(t + 1) * 16],
                    in1=mx_sb[:, t:t + 1].to_broadcast([P, 16]),
                    op=mybir.AluOpType.subtract)
            ex_sb = pp.tile([P, NT128 * 16], f32)
            nc.scalar.activation(ex_sb[:, :], sh_sb[:, :],
                                 mybir.ActivationFunctionType.Exp)
            for t in range(NT128):
                nc.vector.tensor_reduce(out=sm_sb[:, t:t + 1],
                                        in_=ex_sb[:, t * 16:(t + 1) * 16],
                                        axis=mybir.AxisListType.X,
                                        op=mybir.AluOpType.add)
            nc.scalar.activation(lg_sb[:, :], sm_sb[:, :],
                                 mybir.ActivationFunctionType.Ln)
            res_sb = pp.tile([P, NT128 * 16], f32)
            for t in range(NT128):
                nc.vector.tensor_tensor(
                    out=res_sb[:, t * 16:(t + 1) * 16],
                    in0=sh_sb[:, t * 16:(t + 1) * 16],
                    in1=lg_sb[:, t:t + 1].to_broadcast([P, 16]),
                    op=mybir.AluOpType.subtract)
            nc.sync.dma_start(
                out_ext.rearrange("(t p) c -> p t c", p=P),
                res_sb[:, :].rearrange("p (t c) -> p t c", c=16))

    nc.compile()
    return nc


def prepare(x, W1, W2, edge_src, edge_dst):
    """Compile (cached) and build per-core input maps."""
    x = np.asarray(x, dtype=np.float32)
    W1 = np.asarray(W1, dtype=np.float32)
    W2 = np.asarray(W2, dtype=np.float32)

    deg_full, per_core = _host_prep(edge_src, edge_dst)

    nidx_list = [max(pc["nidx_list"][t] for pc in per_core)
                 for t in range(N_TILES)]
    nb_list = [max(pc["nb_list"][t] for pc in per_core) for t in range(N_TILES)]

    key = (tuple(nidx_list), tuple(nb_list))
    if key not in _COMPILED_CACHE:
        _COMPILED_CACHE[key] = _build_program(nidx_list, nb_list)
    nc = _COMPILED_CACHE[key]

    e16 = np.tile(np.eye(16, dtype=np.float32), (8, 1))
    ident = np.eye(16, dtype=np.float32)
    xT = np.ascontiguousarray(x.T)  # [512, N]
    scale_full = (1.0 - ALPHA) / np.maximum(deg_full, 1e-12)

    in_maps = []
    for c in range(NCORES):
        pc = per_core[c]
        xt_shard = np.zeros((N_FEAT, SHARD_PAD), dtype=np.float32)
        xt_shard[:, :SHARD] = xT[:, c * SHARD:(c + 1) * SHARD]
        scale16 = np.tile(
            np.pad(scale_full[c * SHARD:(c + 1) * SHARD],
                   (0, SHARD_PAD - SHARD), constant_values=1.0)[None, :],
            (16, 1)).astype(np.float32)
        idx_cat = np.concatenate(
            [np.pad(pc["idx_tiles"][t],
                    ((0, 0), (0, (nidx_list[t] - pc["nidx_list"][t]) // 16)),
                    constant_values=SHARD)
             for t in range(N_TILES)], axis=1)
        bnd_cat = np.concatenate(
            [np.pad(pc["bnd_tiles"][t],
                    ((0, 0), (0, (nb_list[t] - pc["nb_list"][t]) // 16)))
             for t in range(N_TILES)], axis=1)
        in_maps.append({
            "xt_shard": xt_shard,
            "w1": W1,
            "w2": W2,
            "scale16": scale16,
            "e16": e16,
            "ident": ident,
            "gidx": idx_cat,
            "gbnd": bnd_cat,
        })

    return nc, in_maps


def kernel(x, W1, W2, edge_src, edge_dst):
    from concourse.bass_utils import run_bass_kernel_spmd

    nc, in_maps = prepare(x, W1, W2, edge_src, edge_dst)
    res = run_bass_kernel_spmd(nc, in_maps, list(range(NCORES)))

    out = np.empty((N_NODES, N_CLASS), dtype=np.float32)
    for c in range(NCORES):
        out[c * SHARD:(c + 1) * SHARD] = res.results[c]["out"][:SHARD]
    return out
